# revision 10
# baseline (speedup 1.0000x reference)
"""Multi-head dilated attention on 8 trn2 NeuronCores.

Algorithm restructure (validated vs reference in fp32):
  The reference's score is diagonal: each dilated position t in a segment
  scores q_t.k_t, softmax runs over the segment's sd=128 dilated positions,
  and the output at t is probs_t * v_t.  A token at segment position p only
  feeds heads h with h % 4 == p % 4, so for each "class" c = p % 4 the whole
  module collapses to dense matmuls over that class's 256 channel columns:
    Qc = x_c @ Wq[:, cols_c]   (same K, V), scores = rowwise head dots,
    softmax over the 128 positions, y = (probs*Vc) @ Wo[cols_c, :].
  This is 1/4 the projection FLOPs of the naive form.

Sharding: 8 cores = 4 classes x 2 halves.  Core k owns class k//2 and
batches [ (k%2)*2, (k%2)*2+2 ) -- 16 (batch,segment) units = 2048 tokens.
No collectives; host gathers/scatters.
"""
import numpy as np

B, S, D = 4, 4096, 1024
H, HD, SEG, R = 16, 64, 512, 4
N_SEG = S // SEG          # 8
SD = SEG // R             # 128 dilated positions per segment
SCALE = HD ** -0.5
N_CORES = 8
UNITS_PER_CORE = 16       # (batch, segment) pairs
TOK = UNITS_PER_CORE * SD # 2048 tokens per core
TB = 4                    # token blocks of 512 (4 units each)
TBLK = 512
NK = D // 128             # 8 contraction chunks

_NC_CACHE = {}


def _build_nc(has_bqkv):
    import concourse.bass as bass
    import concourse.mybir as mybir
    import concourse.bacc as bacc
    from concourse import tile
    from contextlib import ExitStack

    f32 = mybir.dt.float32
    f32r = mybir.dt.float32r
    AF = mybir.ActivationFunctionType

    nc = bacc.Bacc("TRN2", target_bir_lowering=False, debug=False)

    xh = nc.dram_tensor("xh", [128, TB, NK, TBLK], f32, kind="ExternalInput").ap()
    wh = nc.dram_tensor("wh", [128, NK, 768], f32, kind="ExternalInput").ap()
    woh = nc.dram_tensor("woh", [128, 2, D], f32, kind="ExternalInput").ap()
    smask = nc.dram_tensor("smask", [128, 8], f32, kind="ExternalInput").ap()
    emask = nc.dram_tensor("emask", [4, 256], f32, kind="ExternalInput").ap()
    bqkv = None
    if has_bqkv:
        bqkv = nc.dram_tensor("bqkv", [128, 6], f32, kind="ExternalInput").ap()
    y = nc.dram_tensor("y", [TOK, D], f32, kind="ExternalOutput").ap()

    with ExitStack() as ctx:
        tc = ctx.enter_context(tile.TileContext(nc))
        cpool = ctx.enter_context(tc.tile_pool(name="const", bufs=1))
        xpool = ctx.enter_context(tc.tile_pool(name="x", bufs=1))
        wk = ctx.enter_context(tc.tile_pool(name="work", bufs=2))
        ps_qkv = ctx.enter_context(tc.tile_pool(name="ps_qkv", bufs=2, space="PSUM"))
        ps_sc = ctx.enter_context(tc.tile_pool(name="ps_sc", bufs=1, space="PSUM"))
        ps_ex = ctx.enter_context(tc.tile_pool(name="ps_ex", bufs=1, space="PSUM"))
        ps_y = ctx.enter_context(tc.tile_pool(name="ps_y", bufs=2, space="PSUM"))

        # PE warm-up: ~8us of throwaway matmuls during the DMA head keeps
        # the HAM activity monitor from starting the real work at 1.2 GHz
        warm = cpool.tile([128, TBLK], f32)
        nc.vector.memset(warm[:], 0.0)
        nc.scalar.activation(warm[:1, 1:2], warm[:1, 0:1], AF.Exp)
        ps_w = ps_sc.tile([128, TBLK], f32, tag="sc")
        for i in range(12):
            nc.tensor.matmul(ps_w[:], lhsT=warm[:, 0:128], rhs=warm[:],
                             start=(i == 0), stop=(i == 11))

        sm_sb = cpool.tile([128, 8], f32r)
        nc.gpsimd.dma_start(sm_sb[:], smask.bitcast(f32r))
        em_sb = cpool.tile([4, 256], f32r)
        nc.gpsimd.dma_start(em_sb[:], emask.bitcast(f32r))
        if has_bqkv:
            bq_sb = cpool.tile([128, 6], f32)
            nc.gpsimd.dma_start(bq_sb[:], bqkv)
        # Bulk input DMA rides ONLY the sync + gpsimd queues (the scalar
        # queue stays clean for ACT evacuations -- a ring-full dma_start
        # would stall them).  Per-engine FIFO within a queue gives strict
        # priority: w and x[tb0] (needed first, in full) lead both queues,
        # split k-wise so both queues carry ~equal bytes.
        w_sb = cpool.tile([128, NK, 768], f32r)
        x_sb = xpool.tile([128, TB, NK, TBLK], f32r)
        wo_sb = cpool.tile([128, 2, D], f32r)
        nc.sync.dma_start(w_sb[:, 0:4, :], wh[:, 0:4, :].bitcast(f32r))
        nc.gpsimd.dma_start(w_sb[:, 4:8, :], wh[:, 4:8, :].bitcast(f32r))
        nc.sync.dma_start(x_sb[:, 0, 0:4], xh[:, 0, 0:4].bitcast(f32r))
        nc.gpsimd.dma_start(x_sb[:, 0, 4:8], xh[:, 0, 4:8].bitcast(f32r))
        nc.sync.dma_start(wo_sb[:], woh.bitcast(f32r))
        for tb in range(1, TB):
            nc.sync.dma_start(x_sb[:, tb, 0:4], xh[:, tb, 0:4].bitcast(f32r))
            nc.gpsimd.dma_start(x_sb[:, tb, 4:8], xh[:, tb, 4:8].bitcast(f32r))

        for tb in range(TB):
            t0 = tb * TBLK
            q_t = []
            qk_t = []
            v_t = []
            # m: 0,1 = Q chunks; 2,3 = K; 4,5 = V.  V is issued after the
            # score matmuls so the PE has work during the softmax chain.
            def qkv_mm(m):
                ps = ps_qkv.tile([128, TBLK], f32, tag="qkv")
                for k in range(NK):
                    nc.tensor.matmul(
                        ps[:],
                        lhsT=w_sb[:, k, m * 128:(m + 1) * 128],
                        rhs=x_sb[:, tb, k, :],
                        start=(k == 0), stop=(k == NK - 1),
                    )
                return ps

            for m in range(2):
                ps = qkv_mm(m)
                q = wk.tile([128, TBLK], f32, tag="q", bufs=4)
                if has_bqkv:
                    nc.scalar.activation(q[:], ps[:], AF.Identity,
                                         bias=bq_sb[:, m:m + 1])
                else:
                    nc.scalar.copy(q[:], ps[:])
                q_t.append(q)
            for m in range(2, 4):
                ps = qkv_mm(m)
                qk = wk.tile([128, TBLK], f32r, tag="qk", bufs=4)
                if has_bqkv:
                    kk = wk.tile([128, TBLK], f32, tag="kk", bufs=2)
                    nc.scalar.activation(kk[:], ps[:], AF.Identity,
                                         bias=bq_sb[:, m:m + 1])
                    nc.vector.tensor_mul(qk[:], q_t[m - 2][:], kk[:])
                else:
                    nc.vector.tensor_mul(qk[:], q_t[m - 2][:], ps[:])
                qk_t.append(qk)

            # scores: per-head partition-group sums via mask matmul (fp32)
            ps_s = ps_sc.tile([4, TBLK], f32, tag="sc")
            nc.tensor.matmul(ps_s[:], lhsT=sm_sb[:, 0:4], rhs=qk_t[0][:],
                             start=True, stop=False)
            nc.tensor.matmul(ps_s[:], lhsT=sm_sb[:, 4:8], rhs=qk_t[1][:],
                             start=False, stop=True)

            for m in range(4, 6):
                ps = qkv_mm(m)
                v = wk.tile([128, TBLK], f32, tag="v", bufs=4)
                if has_bqkv:
                    nc.scalar.activation(v[:], ps[:], AF.Identity,
                                         bias=bq_sb[:, m:m + 1])
                else:
                    nc.scalar.copy(v[:], ps[:])
                v_t.append(v)

            # softmax over the 128 tokens of each unit (scores are O(1),
            # so no max subtraction -- exp is safe in fp32)
            e = wk.tile([4, TBLK], f32r, tag="e", bufs=2)
            nc.scalar.activation(e[:], ps_s[:], AF.Exp)
            sums = wk.tile([4, 4], f32, tag="sums", bufs=2)
            nc.vector.reduce_sum(sums[:], e[:].rearrange("p (u t) -> p u t", u=4),
                                 axis=mybir.AxisListType.X)
            recip = wk.tile([4, 4], f32, tag="recip", bufs=2)
            nc.vector.reciprocal(recip[:], sums[:])
            for u in range(4):
                nc.vector.tensor_scalar_mul(
                    e[:, u * SD:(u + 1) * SD], e[:, u * SD:(u + 1) * SD],
                    recip[:, u:u + 1])

            # expand probs to channel rows, multiply with V
            attn = []
            for i in range(2):
                pse = ps_ex.tile([128, TBLK], f32, tag="ex")
                nc.tensor.matmul(pse[:], lhsT=em_sb[:, i * 128:(i + 1) * 128],
                                 rhs=e[:], start=True, stop=True)
                a = wk.tile([128, TBLK], f32r, tag=f"attn{i}", bufs=2)
                nc.vector.tensor_mul(a[:], v_t[i][:], pse[:])
                attn.append(a)

            # out projection: y[128 tok, 1024] per sub-block
            for sub in range(4):
                psy = ps_y.tile([128, D], f32, tag="y")
                for kc in range(2):
                    for nh in range(2):
                        nc.tensor.matmul(
                            psy[:, nh * 512:(nh + 1) * 512],
                            lhsT=attn[kc][:, sub * 128:(sub + 1) * 128],
                            rhs=wo_sb[:, kc, nh * 512:(nh + 1) * 512],
                            start=(kc == 0), stop=(kc == 1),
                        )
                ysb = wk.tile([128, D], f32, tag="ysb", bufs=3)
                nc.scalar.copy(ysb[:], psy[:])
                row = (tb * 4 + sub) * 128
                eng = nc.sync if sub % 2 == 0 else nc.gpsimd
                eng.dma_start(y[row:row + 128, :], ysb[:])

    nc.compile()
    return nc


def _host_prep(x, Wq, bq, Wk, bk, Wv, bv, Wo):
    """Per-core input maps."""
    has_bqkv = bool(np.any(bq) or np.any(bk) or np.any(bv))
    xr = np.ascontiguousarray(
        x.reshape(B, N_SEG, SD, R, D).transpose(3, 0, 1, 2, 4))  # (R,B,n,sd,D)
    in_maps = []
    for core in range(N_CORES):
        c, half = core // 2, core % 2
        heads = [c + R * j for j in range(4)]
        cols = np.concatenate([np.arange(h * HD, (h + 1) * HD) for h in heads])
        wqkv = np.ascontiguousarray(
            np.concatenate([Wq[:, cols], Wk[:, cols], Wv[:, cols]], axis=1))
        wo_c = np.ascontiguousarray(Wo[cols, :])
        xc = xr[c, half * 2:half * 2 + 2].reshape(TOK, D)  # (2048, 1024)
        # (128 part, TB, NK, TBLK): per-partition-contiguous per token block
        xhh = np.ascontiguousarray(
            xc.T.reshape(NK, 128, TB, TBLK).transpose(1, 2, 0, 3))

        sm = np.zeros((128, 8), np.float32)
        em = np.zeros((4, 256), np.float32)
        p = np.arange(128)
        for g in range(2):
            sm[p[g * 64:(g + 1) * 64], g] = SCALE
            sm[p[g * 64:(g + 1) * 64], 4 + 2 + g] = SCALE
            em[g, g * 64:(g + 1) * 64] = 1.0
            em[2 + g, 128 + g * 64:128 + (g + 1) * 64] = 1.0
        whh = np.ascontiguousarray(
            wqkv.reshape(NK, 128, 768).transpose(1, 0, 2))
        wohh = np.ascontiguousarray(
            wo_c.reshape(2, 128, D).transpose(1, 0, 2))
        m = {"xh": xhh, "wh": whh, "woh": wohh, "smask": sm, "emask": em}
        if has_bqkv:
            bq_c, bk_c, bv_c = bq[cols], bk[cols], bv[cols]
            m["bqkv"] = np.ascontiguousarray(np.stack(
                [bq_c[:128], bq_c[128:], bk_c[:128], bk_c[128:],
                 bv_c[:128], bv_c[128:]], axis=1)).astype(np.float32)
        in_maps.append(m)
    return in_maps, has_bqkv


def kernel(x, Wq, bq, Wk, bk, Wv, bv, Wo, bo, _trace=False, _trace_kwargs=None):
    from concourse.bass_utils import run_bass_kernel_spmd

    in_maps, has_bqkv = _host_prep(
        np.asarray(x, np.float32), np.asarray(Wq, np.float32),
        np.asarray(bq, np.float32), np.asarray(Wk, np.float32),
        np.asarray(bk, np.float32), np.asarray(Wv, np.float32),
        np.asarray(bv, np.float32), np.asarray(Wo, np.float32))

    key = has_bqkv
    if key not in _NC_CACHE:
        _NC_CACHE[key] = _build_nc(has_bqkv)
    nc = _NC_CACHE[key]

    kwargs = {}
    if _trace:
        kwargs = dict(trace=True, **(_trace_kwargs or {}))
    res = run_bass_kernel_spmd(nc, in_maps, list(range(N_CORES)), **kwargs)

    out = np.zeros((R, B, N_SEG, SD, D), np.float32)
    for core in range(N_CORES):
        c, half = core // 2, core % 2
        out[c, half * 2:half * 2 + 2] = \
            res.results[core]["y"].reshape(2, N_SEG, SD, D)
    out = np.ascontiguousarray(out.transpose(1, 2, 3, 0, 4)).reshape(B, S, D)
    bo = np.asarray(bo, np.float32)
    if np.any(bo):
        out += bo
    if _trace:
        kernel._last_results = res
    return out


# revision 11
# speedup vs baseline: 1.1284x; 1.1284x over previous
"""Multi-head dilated attention on 8 trn2 NeuronCores.

Algorithm restructure (validated vs reference in fp32):
  The reference's score is diagonal: each dilated position t in a segment
  scores q_t.k_t, softmax runs over the segment's sd=128 dilated positions,
  and the output at t is probs_t * v_t.  A token at segment position p only
  feeds heads h with h % 4 == p % 4, so for each "class" c = p % 4 the whole
  module collapses to dense matmuls over that class's 256 channel columns:
    Qc = x_c @ Wq[:, cols_c]   (same K, V), scores = rowwise head dots,
    softmax over the 128 positions, y = (probs*Vc) @ Wo[cols_c, :].
  This is 1/4 the projection FLOPs of the naive form.

Sharding: 8 cores = 4 classes x 2 halves.  Core k owns class k//2 and
batches [ (k%2)*2, (k%2)*2+2 ) -- 16 (batch,segment) units = 2048 tokens.
No collectives; host gathers/scatters.
"""
import numpy as np

B, S, D = 4, 4096, 1024
H, HD, SEG, R = 16, 64, 512, 4
N_SEG = S // SEG          # 8
SD = SEG // R             # 128 dilated positions per segment
SCALE = HD ** -0.5
N_CORES = 8
UNITS_PER_CORE = 16       # (batch, segment) pairs
TOK = UNITS_PER_CORE * SD # 2048 tokens per core
TB = 4                    # token blocks of 512 (4 units each)
TBLK = 512
NK = D // 128             # 8 contraction chunks

_NC_CACHE = {}


def _build_nc(has_bqkv):
    import concourse.bass as bass
    import concourse.mybir as mybir
    import concourse.bacc as bacc
    from concourse import tile
    from contextlib import ExitStack

    f32 = mybir.dt.float32
    f32r = mybir.dt.float32r
    AF = mybir.ActivationFunctionType

    nc = bacc.Bacc("TRN2", target_bir_lowering=False, debug=False)

    xh = nc.dram_tensor("xh", [128, TB, NK, TBLK], f32, kind="ExternalInput").ap()
    wh = nc.dram_tensor("wh", [128, NK, 768], f32, kind="ExternalInput").ap()
    woh = nc.dram_tensor("woh", [128, 2, D], f32, kind="ExternalInput").ap()
    smask = nc.dram_tensor("smask", [128, 8], f32, kind="ExternalInput").ap()
    emask = nc.dram_tensor("emask", [4, 256], f32, kind="ExternalInput").ap()
    bqkv = None
    if has_bqkv:
        bqkv = nc.dram_tensor("bqkv", [128, 6], f32, kind="ExternalInput").ap()
    y = nc.dram_tensor("y", [TOK, D], f32, kind="ExternalOutput").ap()

    with ExitStack() as ctx:
        tc = ctx.enter_context(tile.TileContext(nc))
        cpool = ctx.enter_context(tc.tile_pool(name="const", bufs=1))
        xpool = ctx.enter_context(tc.tile_pool(name="x", bufs=1))
        wk = ctx.enter_context(tc.tile_pool(name="work", bufs=2))
        ps_qkv = ctx.enter_context(tc.tile_pool(name="ps_qkv", bufs=2, space="PSUM"))
        ps_sc = ctx.enter_context(tc.tile_pool(name="ps_sc", bufs=1, space="PSUM"))
        ps_ex = ctx.enter_context(tc.tile_pool(name="ps_ex", bufs=1, space="PSUM"))
        ps_y = ctx.enter_context(tc.tile_pool(name="ps_y", bufs=2, space="PSUM"))

        # PE warm-up: ~8us of throwaway matmuls during the DMA head keeps
        # the HAM activity monitor from starting the real work at 1.2 GHz
        warm = cpool.tile([128, TBLK], f32)
        nc.vector.memset(warm[:], 0.0)
        nc.scalar.activation(warm[:1, 1:2], warm[:1, 0:1], AF.Exp)
        ps_w = ps_sc.tile([128, TBLK], f32, tag="sc")
        for i in range(14):
            nc.tensor.matmul(ps_w[:], lhsT=warm[:, 0:128], rhs=warm[:],
                             start=(i == 0), stop=(i == 13))

        sm_sb = cpool.tile([128, 8], f32r)
        nc.gpsimd.dma_start(sm_sb[:], smask.bitcast(f32r))
        em_sb = cpool.tile([4, 256], f32r)
        nc.gpsimd.dma_start(em_sb[:], emask.bitcast(f32r))
        if has_bqkv:
            bq_sb = cpool.tile([128, 6], f32)
            nc.gpsimd.dma_start(bq_sb[:], bqkv)
        # Bulk input DMA rides ONLY the sync + gpsimd queues (the scalar
        # queue stays clean for ACT evacuations -- a ring-full dma_start
        # would stall them).  Per-engine FIFO within a queue gives strict
        # priority: w and x[tb0] (needed first, in full) lead both queues,
        # split k-wise so both queues carry ~equal bytes.
        w_sb = cpool.tile([128, NK, 768], f32r)
        x_sb = xpool.tile([128, TB, NK, TBLK], f32r)
        wo_sb = cpool.tile([128, 2, D], f32r)
        nc.sync.dma_start(w_sb[:, 0:4, :], wh[:, 0:4, :].bitcast(f32r))
        nc.scalar.dma_start(w_sb[:, 4:8, :], wh[:, 4:8, :].bitcast(f32r))
        nc.sync.dma_start(x_sb[:, 0, 0:4], xh[:, 0, 0:4].bitcast(f32r))
        nc.scalar.dma_start(x_sb[:, 0, 4:8], xh[:, 0, 4:8].bitcast(f32r))
        nc.sync.dma_start(wo_sb[:], woh.bitcast(f32r))
        for tb in range(1, TB):
            nc.sync.dma_start(x_sb[:, tb], xh[:, tb].bitcast(f32r))

        for tb in range(TB):
            t0 = tb * TBLK
            q_t = []
            qk_t = []
            v_t = []
            # m: 0,1 = Q chunks; 2,3 = K; 4,5 = V.  V is issued after the
            # score matmuls so the PE has work during the softmax chain.
            def qkv_mm(m):
                ps = ps_qkv.tile([128, TBLK], f32, tag="qkv")
                for k in range(NK):
                    nc.tensor.matmul(
                        ps[:],
                        lhsT=w_sb[:, k, m * 128:(m + 1) * 128],
                        rhs=x_sb[:, tb, k, :],
                        start=(k == 0), stop=(k == NK - 1),
                    )
                return ps

            for m in range(2):
                ps = qkv_mm(m)
                q = wk.tile([128, TBLK], f32, tag="q", bufs=4)
                if has_bqkv:
                    nc.scalar.activation(q[:], ps[:], AF.Identity,
                                         bias=bq_sb[:, m:m + 1])
                else:
                    nc.scalar.copy(q[:], ps[:])
                q_t.append(q)
            for m in range(2, 4):
                ps = qkv_mm(m)
                qk = wk.tile([128, TBLK], f32r, tag="qk", bufs=4)
                if has_bqkv:
                    kk = wk.tile([128, TBLK], f32, tag="kk", bufs=2)
                    nc.scalar.activation(kk[:], ps[:], AF.Identity,
                                         bias=bq_sb[:, m:m + 1])
                    nc.vector.tensor_mul(qk[:], q_t[m - 2][:], kk[:])
                else:
                    nc.vector.tensor_mul(qk[:], q_t[m - 2][:], ps[:])
                qk_t.append(qk)

            # scores: per-head partition-group sums via mask matmul (fp32)
            ps_s = ps_sc.tile([4, TBLK], f32, tag="sc")
            nc.tensor.matmul(ps_s[:], lhsT=sm_sb[:, 0:4], rhs=qk_t[0][:],
                             start=True, stop=False)
            nc.tensor.matmul(ps_s[:], lhsT=sm_sb[:, 4:8], rhs=qk_t[1][:],
                             start=False, stop=True)

            for m in range(4, 6):
                ps = qkv_mm(m)
                v = wk.tile([128, TBLK], f32, tag="v", bufs=4)
                if has_bqkv:
                    nc.scalar.activation(v[:], ps[:], AF.Identity,
                                         bias=bq_sb[:, m:m + 1])
                else:
                    nc.scalar.copy(v[:], ps[:])
                v_t.append(v)

            # softmax over the 128 tokens of each unit (scores are O(1),
            # so no max subtraction -- exp is safe in fp32)
            e = wk.tile([4, TBLK], f32r, tag="e", bufs=2)
            nc.scalar.activation(e[:], ps_s[:], AF.Exp)
            sums = wk.tile([4, 4], f32, tag="sums", bufs=2)
            nc.vector.reduce_sum(sums[:], e[:].rearrange("p (u t) -> p u t", u=4),
                                 axis=mybir.AxisListType.X)
            recip = wk.tile([4, 4], f32, tag="recip", bufs=2)
            nc.vector.reciprocal(recip[:], sums[:])
            for u in range(4):
                nc.vector.tensor_scalar_mul(
                    e[:, u * SD:(u + 1) * SD], e[:, u * SD:(u + 1) * SD],
                    recip[:, u:u + 1])

            # expand probs to channel rows, multiply with V
            attn = []
            for i in range(2):
                pse = ps_ex.tile([128, TBLK], f32, tag="ex")
                nc.tensor.matmul(pse[:], lhsT=em_sb[:, i * 128:(i + 1) * 128],
                                 rhs=e[:], start=True, stop=True)
                a = wk.tile([128, TBLK], f32r, tag=f"attn{i}", bufs=2)
                nc.vector.tensor_mul(a[:], v_t[i][:], pse[:])
                attn.append(a)

            # out projection: y[128 tok, 1024] per sub-block
            for sub in range(4):
                psy = ps_y.tile([128, D], f32, tag="y")
                for kc in range(2):
                    for nh in range(2):
                        nc.tensor.matmul(
                            psy[:, nh * 512:(nh + 1) * 512],
                            lhsT=attn[kc][:, sub * 128:(sub + 1) * 128],
                            rhs=wo_sb[:, kc, nh * 512:(nh + 1) * 512],
                            start=(kc == 0), stop=(kc == 1),
                        )
                ysb = wk.tile([128, D], f32, tag="ysb", bufs=3)
                nc.scalar.copy(ysb[:], psy[:])
                row = (tb * 4 + sub) * 128
                if tb == TB - 1:
                    eng = nc.sync
                else:
                    eng = nc.sync if sub % 2 == 0 else nc.gpsimd
                eng.dma_start(y[row:row + 128, :], ysb[:])

    nc.compile()
    return nc


def _host_prep(x, Wq, bq, Wk, bk, Wv, bv, Wo):
    """Per-core input maps."""
    has_bqkv = bool(np.any(bq) or np.any(bk) or np.any(bv))
    xr = np.ascontiguousarray(
        x.reshape(B, N_SEG, SD, R, D).transpose(3, 0, 1, 2, 4))  # (R,B,n,sd,D)
    in_maps = []
    for core in range(N_CORES):
        c, half = core // 2, core % 2
        heads = [c + R * j for j in range(4)]
        cols = np.concatenate([np.arange(h * HD, (h + 1) * HD) for h in heads])
        wqkv = np.ascontiguousarray(
            np.concatenate([Wq[:, cols], Wk[:, cols], Wv[:, cols]], axis=1))
        wo_c = np.ascontiguousarray(Wo[cols, :])
        xc = xr[c, half * 2:half * 2 + 2].reshape(TOK, D)  # (2048, 1024)
        # (128 part, TB, NK, TBLK): per-partition-contiguous per token block
        xhh = np.ascontiguousarray(
            xc.T.reshape(NK, 128, TB, TBLK).transpose(1, 2, 0, 3))

        sm = np.zeros((128, 8), np.float32)
        em = np.zeros((4, 256), np.float32)
        p = np.arange(128)
        for g in range(2):
            sm[p[g * 64:(g + 1) * 64], g] = SCALE
            sm[p[g * 64:(g + 1) * 64], 4 + 2 + g] = SCALE
            em[g, g * 64:(g + 1) * 64] = 1.0
            em[2 + g, 128 + g * 64:128 + (g + 1) * 64] = 1.0
        whh = np.ascontiguousarray(
            wqkv.reshape(NK, 128, 768).transpose(1, 0, 2))
        wohh = np.ascontiguousarray(
            wo_c.reshape(2, 128, D).transpose(1, 0, 2))
        m = {"xh": xhh, "wh": whh, "woh": wohh, "smask": sm, "emask": em}
        if has_bqkv:
            bq_c, bk_c, bv_c = bq[cols], bk[cols], bv[cols]
            m["bqkv"] = np.ascontiguousarray(np.stack(
                [bq_c[:128], bq_c[128:], bk_c[:128], bk_c[128:],
                 bv_c[:128], bv_c[128:]], axis=1)).astype(np.float32)
        in_maps.append(m)
    return in_maps, has_bqkv


def kernel(x, Wq, bq, Wk, bk, Wv, bv, Wo, bo, _trace=False, _trace_kwargs=None):
    from concourse.bass_utils import run_bass_kernel_spmd

    in_maps, has_bqkv = _host_prep(
        np.asarray(x, np.float32), np.asarray(Wq, np.float32),
        np.asarray(bq, np.float32), np.asarray(Wk, np.float32),
        np.asarray(bk, np.float32), np.asarray(Wv, np.float32),
        np.asarray(bv, np.float32), np.asarray(Wo, np.float32))

    key = has_bqkv
    if key not in _NC_CACHE:
        _NC_CACHE[key] = _build_nc(has_bqkv)
    nc = _NC_CACHE[key]

    kwargs = {}
    if _trace:
        kwargs = dict(trace=True, **(_trace_kwargs or {}))
    res = run_bass_kernel_spmd(nc, in_maps, list(range(N_CORES)), **kwargs)

    out = np.zeros((R, B, N_SEG, SD, D), np.float32)
    for core in range(N_CORES):
        c, half = core // 2, core % 2
        out[c, half * 2:half * 2 + 2] = \
            res.results[core]["y"].reshape(2, N_SEG, SD, D)
    out = np.ascontiguousarray(out.transpose(1, 2, 3, 0, 4)).reshape(B, S, D)
    bo = np.asarray(bo, np.float32)
    if np.any(bo):
        out += bo
    if _trace:
        kernel._last_results = res
    return out
